# revision 10
# baseline (speedup 1.0000x reference)
"""Depthwise-separable conv2d block (dw3x3 + BN + ReLU + map-cut, pw1x1 + BN +
ReLU + map-cut) on 8 Trainium2 NeuronCores, data-parallel over the batch dim.

Fixed problem shapes: x (32,256,56,56) f32 -> out (32,512,54,54) f32.

Per-core device program (4 images each):
  - depthwise 3x3 VALID conv as 9 PSUM-accumulated diagonal matmuls per
    486-column chunk (fp32r, full PE rate at N>=256)
  - fused PSUM->SBUF copy + running per-map max via tensor_tensor_reduce
  - per-map cut mask + BN-folded bias + ReLU in one scalar-engine pass:
    y = relu(conv * mask + (bias * mask))
  - pointwise 1x1 conv as K=256 GEMM (fp32r), same fused max/copy, same
    fused mask+bias+ReLU pass, then DMA out.
BatchNorm (inference) is folded into the conv weights/biases on the host.
"""

import ml_dtypes
import numpy as np

import concourse.bacc as bacc
import concourse.bass as bass
import concourse.mybir as mybir
import concourse.tile as tile
from concourse.bass_utils import run_bass_kernel_spmd

EPS = 1e-5
DW_THRESH = 4.0
PW_THRESH = 0.001

B, CIN, COUT, H, W = 32, 256, 512, 56, 56
HO, WO = 54, 54
NPIX = HO * WO          # 2916
NCORES = 8
BPC = B // NCORES       # 4 images per core
P = 128                 # partitions
KT = CIN // P           # 2 cin tiles
MT = COUT // P          # 4 cout tiles
NCH = 6                 # output chunks per map: 6 x (9 rows x 54 cols)
CHROWS = HO // NCH      # 9
CHUNK = CHROWS * WO     # 486 columns per chunk (one PSUM bank)
NEG = -3.0e38

F32 = mybir.dt.float32
F32R = mybir.dt.float32r
BF16 = mybir.dt.bfloat16

_cached_nc = None


def _build_program():
    nc = bacc.Bacc("TRN2", target_bir_lowering=False, debug=False)

    xs = nc.dram_tensor("xs", [BPC, CIN, H, W], BF16, kind="ExternalInput").ap()
    dwdiag = nc.dram_tensor("dwdiag", [P, KT, 9, P], BF16, kind="ExternalInput").ap()
    w2t = nc.dram_tensor("w2t", [P, KT, COUT], BF16, kind="ExternalInput").ap()
    b1t = nc.dram_tensor("b1t", [P, KT], F32, kind="ExternalInput").ap()
    b2t = nc.dram_tensor("b2t", [P, MT], F32, kind="ExternalInput").ap()
    zs = nc.dram_tensor("zs", [BPC, COUT, HO, WO], F32, kind="ExternalOutput").ap()
    zs_flat = zs.rearrange("b c h w -> b c (h w)")

    HCH = NCH // 2          # 3 chunks per PSUM half-group
    BANK = 512              # fp32 elems per PSUM bank

    with tile.TileContext(nc) as tc:
        with (
            tc.tile_pool(name="consts", bufs=1) as consts,
            tc.tile_pool(name="xp", bufs=8) as xp,
            tc.tile_pool(name="yp", bufs=4) as yp,
            tc.tile_pool(name="zp", bufs=4) as zp,
            tc.tile_pool(name="wm", bufs=4) as wmp,
            tc.tile_pool(name="st", bufs=24) as st,
            tc.tile_pool(name="psdw", bufs=2, space="PSUM") as psdw,
            tc.tile_pool(name="pspw", bufs=2, space="PSUM") as pspw,
        ):
            dwsb = consts.tile([P, KT, 9, P], BF16)
            nc.sync.dma_start(out=dwsb, in_=dwdiag)
            w2sb = consts.tile([P, KT, COUT], BF16)
            nc.sync.dma_start(out=w2sb, in_=w2t)
            b1sb = consts.tile([P, KT], F32)
            nc.sync.dma_start(out=b1sb, in_=b1t)
            b2sb = consts.tile([P, MT], F32)
            nc.sync.dma_start(out=b2sb, in_=b2t)

            xtiles = {}
            for b in range(BPC):
                for k in range(KT):
                    X = xp.tile([P, H, W], BF16)
                    nc.gpsimd.dma_start(out=X, in_=xs[b, k * P:(k + 1) * P, :, :])
                    xtiles[b, k] = X

            for b in range(BPC):
                ytiles = []
                wmtiles = []
                for k in range(KT):
                    X = xtiles[b, k]

                    Y = yp.tile([P, NPIX], BF16)
                    Y3 = Y.rearrange("p (c x) -> p c x", x=CHUNK)
                    mx = st.tile([P, NCH], F32)
                    for h in range(2):
                        # 3 PSUM banks hold 3 conv chunks (27 rows of output)
                        P1 = psdw.tile([P, HCH, BANK], F32)
                        for j in range(HCH):
                            n = h * HCH + j
                            for t in range(9):
                                di, dj = t // 3, t % 3
                                rhs = X[:,
                                        CHROWS * n + di: CHROWS * n + di + CHROWS,
                                        dj: dj + WO]
                                nc.tensor.matmul(
                                    P1[:, j, 0:CHUNK],
                                    lhsT=dwsb[:, k, t, :],
                                    rhs=rhs,
                                    start=(t == 0),
                                    stop=(t == 8),
                                )
                        # one strided max over the 3 chunks in this half-group
                        nc.vector.tensor_reduce(
                            mx[:, h * HCH:(h + 1) * HCH], P1[:, :, 0:CHUNK],
                            axis=mybir.AxisListType.X, op=mybir.AluOpType.max)
                        # mask-free relu+bias straight away: PSUM frees early.
                        # the dw cut is applied later via masked pw weights.
                        nc.scalar.activation(
                            out=Y3[:, h * HCH:(h + 1) * HCH, :],
                            in_=P1[:, :, 0:CHUNK],
                            func=mybir.ActivationFunctionType.Relu,
                            bias=b1sb[:, k:k + 1], scale=1.0)
                    m1 = st.tile([P, 1], F32)
                    nc.vector.tensor_reduce(
                        m1, mx, axis=mybir.AxisListType.X, op=mybir.AluOpType.max)
                    nc.vector.tensor_tensor(
                        m1, m1, b1sb[:, k:k + 1], op=mybir.AluOpType.add)
                    mask1 = st.tile([P, 1], F32)
                    nc.vector.tensor_scalar(
                        out=mask1, in0=m1, scalar1=DW_THRESH, scalar2=None,
                        op0=mybir.AluOpType.is_ge)
                    # fold the per-(image,channel) dw cut into the pw weights
                    w2m = wmp.tile([P, COUT], BF16)
                    nc.vector.tensor_scalar(
                        out=w2m, in0=w2sb[:, k, :], scalar1=mask1, scalar2=None,
                        op0=mybir.AluOpType.mult)
                    ytiles.append(Y)
                    wmtiles.append(w2m)

                for m in range(MT):
                    Z = zp.tile([P, NPIX], F32)
                    mzx = st.tile([P, NCH], F32)
                    for n in range(NCH):
                        P2 = pspw.tile([P, CHUNK], F32)
                        for k in range(KT):
                            nc.tensor.matmul(
                                P2,
                                lhsT=wmtiles[k][:, m * P:(m + 1) * P],
                                rhs=ytiles[k][:, n * CHUNK:(n + 1) * CHUNK],
                                start=(k == 0),
                                stop=(k == KT - 1),
                            )
                        nc.vector.tensor_reduce(
                            mzx[:, n:n + 1], P2,
                            axis=mybir.AxisListType.X, op=mybir.AluOpType.max)
                        # copy chunk out of PSUM with the bias folded in
                        nc.scalar.activation(
                            out=Z[:, n * CHUNK:(n + 1) * CHUNK], in_=P2,
                            func=mybir.ActivationFunctionType.Identity,
                            bias=b2sb[:, m:m + 1], scale=1.0)
                    m2 = st.tile([P, 1], F32)
                    nc.vector.tensor_reduce(
                        m2, mzx, axis=mybir.AxisListType.X, op=mybir.AluOpType.max)
                    nc.vector.tensor_tensor(
                        m2, m2, b2sb[:, m:m + 1], op=mybir.AluOpType.add)
                    mask2 = st.tile([P, 1], F32)
                    nc.vector.tensor_scalar(
                        out=mask2, in0=m2, scalar1=PW_THRESH, scalar2=None,
                        op0=mybir.AluOpType.is_ge)
                    # z = max(z*mask, 0): alternate DVE / ACT to balance engines
                    if m % 2 == 0:
                        nc.vector.tensor_scalar(
                            out=Z, in0=Z, scalar1=mask2, scalar2=0.0,
                            op0=mybir.AluOpType.mult, op1=mybir.AluOpType.max)
                    else:
                        nc.scalar.activation(
                            out=Z, in_=Z,
                            func=mybir.ActivationFunctionType.Relu,
                            bias=0.0, scale=mask2)
                    nc.sync.dma_start(
                        out=zs_flat[b, m * P:(m + 1) * P, :], in_=Z)
    nc.compile()
    return nc


def _prep_params(dw_w, dw_b, dw_gamma, dw_beta, dw_mean, dw_var,
                 pw_w, pw_b, pw_gamma, pw_beta, pw_mean, pw_var):
    dw_scale = dw_gamma / np.sqrt(dw_var + EPS)
    b1 = dw_b * dw_scale + dw_beta - dw_mean * dw_scale          # (256,)
    w1 = dw_w[:, 0] * dw_scale[:, None, None]                    # (256,3,3)

    dwdiag = np.zeros((P, KT, 9, P), np.float32)
    idx = np.arange(P)
    for k in range(KT):
        for t in range(9):
            dwdiag[idx, k, t, idx] = w1[k * P:(k + 1) * P, t // 3, t % 3]

    pw_scale = pw_gamma / np.sqrt(pw_var + EPS)
    b2 = pw_b * pw_scale + pw_beta - pw_mean * pw_scale          # (512,)
    w2 = pw_w * pw_scale[:, None]                                # (512,256)
    # w2t[ck, k, o] = w2[o, k*128+ck]
    w2t = np.ascontiguousarray(
        w2.T.reshape(KT, P, COUT).transpose(1, 0, 2)).astype(np.float32)
    b1t = np.ascontiguousarray(b1.reshape(KT, P).T).astype(np.float32)
    b2t = np.ascontiguousarray(b2.reshape(MT, P).T).astype(np.float32)
    return (np.ascontiguousarray(dwdiag).astype(ml_dtypes.bfloat16),
            w2t.astype(ml_dtypes.bfloat16), b1t, b2t)


def kernel(x, dw_w, dw_b, dw_gamma, dw_beta, dw_mean, dw_var,
           pw_w, pw_b, pw_gamma, pw_beta, pw_mean, pw_var):
    global _cached_nc
    x = np.ascontiguousarray(np.asarray(x, np.float32))
    args = [np.asarray(a, np.float32) for a in
            (dw_w, dw_b, dw_gamma, dw_beta, dw_mean, dw_var,
             pw_w, pw_b, pw_gamma, pw_beta, pw_mean, pw_var)]
    dwdiag, w2t, b1t, b2t = _prep_params(*args)
    x16 = x.astype(ml_dtypes.bfloat16)

    if _cached_nc is None:
        _cached_nc = _build_program()
    nc = _cached_nc

    in_maps = []
    for c in range(NCORES):
        in_maps.append({
            "xs": np.ascontiguousarray(x16[c * BPC:(c + 1) * BPC]),
            "dwdiag": dwdiag,
            "w2t": w2t,
            "b1t": b1t,
            "b2t": b2t,
        })
    res = run_bass_kernel_spmd(nc, in_maps, core_ids=list(range(NCORES)))
    out = np.concatenate([res.results[c]["zs"] for c in range(NCORES)], axis=0)
    return out


# revision 11
# speedup vs baseline: 1.0001x; 1.0001x over previous
"""Depthwise-separable conv2d block (dw3x3 + BN + ReLU + map-cut, pw1x1 + BN +
ReLU + map-cut) on 8 Trainium2 NeuronCores, data-parallel over the batch dim.

Fixed problem shapes: x (32,256,56,56) f32 -> out (32,512,54,54) f32.

Per-core device program (4 images each):
  - depthwise 3x3 VALID conv as 9 PSUM-accumulated diagonal matmuls per
    486-column chunk (fp32r, full PE rate at N>=256)
  - fused PSUM->SBUF copy + running per-map max via tensor_tensor_reduce
  - per-map cut mask + BN-folded bias + ReLU in one scalar-engine pass:
    y = relu(conv * mask + (bias * mask))
  - pointwise 1x1 conv as K=256 GEMM (fp32r), same fused max/copy, same
    fused mask+bias+ReLU pass, then DMA out.
BatchNorm (inference) is folded into the conv weights/biases on the host.
"""

import ml_dtypes
import numpy as np

import concourse.bacc as bacc
import concourse.bass as bass
import concourse.mybir as mybir
import concourse.tile as tile
from concourse.bass_utils import run_bass_kernel_spmd

EPS = 1e-5
DW_THRESH = 4.0
PW_THRESH = 0.001

B, CIN, COUT, H, W = 32, 256, 512, 56, 56
HO, WO = 54, 54
NPIX = HO * WO          # 2916
NCORES = 8
BPC = B // NCORES       # 4 images per core
P = 128                 # partitions
KT = CIN // P           # 2 cin tiles
MT = COUT // P          # 4 cout tiles
NCH = 6                 # output chunks per map: 6 x (9 rows x 54 cols)
CHROWS = HO // NCH      # 9
CHUNK = CHROWS * WO     # 486 columns per chunk (one PSUM bank)
NEG = -3.0e38

F32 = mybir.dt.float32
F32R = mybir.dt.float32r
BF16 = mybir.dt.bfloat16

_cached_nc = None


def _build_program():
    nc = bacc.Bacc("TRN2", target_bir_lowering=False, debug=False)

    xs = nc.dram_tensor("xs", [BPC, CIN, H, W], BF16, kind="ExternalInput").ap()
    dwdiag = nc.dram_tensor("dwdiag", [P, KT, 9, P], BF16, kind="ExternalInput").ap()
    w2t = nc.dram_tensor("w2t", [P, KT, COUT], BF16, kind="ExternalInput").ap()
    b1t = nc.dram_tensor("b1t", [P, KT], F32, kind="ExternalInput").ap()
    b2t = nc.dram_tensor("b2t", [P, MT], F32, kind="ExternalInput").ap()
    zs = nc.dram_tensor("zs", [BPC, COUT, HO, WO], F32, kind="ExternalOutput").ap()
    zs_flat = zs.rearrange("b c h w -> b c (h w)")

    HCH = NCH // 2          # 3 chunks per PSUM half-group
    BANK = 512              # fp32 elems per PSUM bank

    with tile.TileContext(nc) as tc:
        with (
            tc.tile_pool(name="consts", bufs=1) as consts,
            tc.tile_pool(name="xp", bufs=8) as xp,
            tc.tile_pool(name="yp", bufs=4) as yp,
            tc.tile_pool(name="zp", bufs=4) as zp,
            tc.tile_pool(name="wm", bufs=4) as wmp,
            tc.tile_pool(name="st", bufs=24) as st,
            tc.tile_pool(name="psdw", bufs=2, space="PSUM") as psdw,
            tc.tile_pool(name="pspw", bufs=2, space="PSUM") as pspw,
        ):
            # first image's inputs and the dw weights gate the first matmul —
            # issue those DMAs first, the rest of the prefetch afterwards
            xtiles = {}
            for b, k in [(0, 0), (0, 1)]:
                X = xp.tile([P, H, W], BF16)
                nc.sync.dma_start(out=X, in_=xs[b, k * P:(k + 1) * P, :, :])
                xtiles[b, k] = X
            dwsb = consts.tile([P, KT, 9, P], BF16)
            nc.sync.dma_start(out=dwsb, in_=dwdiag)
            w2sb = consts.tile([P, KT, COUT], BF16)
            nc.sync.dma_start(out=w2sb, in_=w2t)
            b1sb = consts.tile([P, KT], F32)
            nc.sync.dma_start(out=b1sb, in_=b1t)
            b2sb = consts.tile([P, MT], F32)
            nc.sync.dma_start(out=b2sb, in_=b2t)
            for b in range(BPC):
                for k in range(KT):
                    if (b, k) in xtiles:
                        continue
                    X = xp.tile([P, H, W], BF16)
                    nc.sync.dma_start(out=X, in_=xs[b, k * P:(k + 1) * P, :, :])
                    xtiles[b, k] = X

            for b in range(BPC):
                ytiles = []
                wmtiles = []
                for k in range(KT):
                    X = xtiles[b, k]

                    Y = yp.tile([P, NPIX], BF16)
                    Y3 = Y.rearrange("p (c x) -> p c x", x=CHUNK)
                    mx = st.tile([P, NCH], F32)
                    for h in range(2):
                        # 3 PSUM banks hold 3 conv chunks (27 rows of output)
                        P1 = psdw.tile([P, HCH, BANK], F32)
                        for j in range(HCH):
                            n = h * HCH + j
                            for t in range(9):
                                di, dj = t // 3, t % 3
                                rhs = X[:,
                                        CHROWS * n + di: CHROWS * n + di + CHROWS,
                                        dj: dj + WO]
                                nc.tensor.matmul(
                                    P1[:, j, 0:CHUNK],
                                    lhsT=dwsb[:, k, t, :],
                                    rhs=rhs,
                                    start=(t == 0),
                                    stop=(t == 8),
                                )
                        # one strided max over the 3 chunks in this half-group
                        nc.vector.tensor_reduce(
                            mx[:, h * HCH:(h + 1) * HCH], P1[:, :, 0:CHUNK],
                            axis=mybir.AxisListType.X, op=mybir.AluOpType.max)
                        # mask-free relu+bias straight away: PSUM frees early.
                        # the dw cut is applied later via masked pw weights.
                        nc.scalar.activation(
                            out=Y3[:, h * HCH:(h + 1) * HCH, :],
                            in_=P1[:, :, 0:CHUNK],
                            func=mybir.ActivationFunctionType.Relu,
                            bias=b1sb[:, k:k + 1], scale=1.0)
                    m1 = st.tile([P, 1], F32)
                    nc.vector.tensor_reduce(
                        m1, mx, axis=mybir.AxisListType.X, op=mybir.AluOpType.max)
                    nc.vector.tensor_tensor(
                        m1, m1, b1sb[:, k:k + 1], op=mybir.AluOpType.add)
                    mask1 = st.tile([P, 1], F32)
                    nc.vector.tensor_scalar(
                        out=mask1, in0=m1, scalar1=DW_THRESH, scalar2=None,
                        op0=mybir.AluOpType.is_ge)
                    # fold the per-(image,channel) dw cut into the pw weights
                    w2m = wmp.tile([P, COUT], BF16)
                    nc.vector.tensor_scalar(
                        out=w2m, in0=w2sb[:, k, :], scalar1=mask1, scalar2=None,
                        op0=mybir.AluOpType.mult)
                    ytiles.append(Y)
                    wmtiles.append(w2m)

                for m in range(MT):
                    Z = zp.tile([P, NPIX], F32)
                    mzx = st.tile([P, NCH], F32)
                    for n in range(NCH):
                        P2 = pspw.tile([P, CHUNK], F32)
                        for k in range(KT):
                            nc.tensor.matmul(
                                P2,
                                lhsT=wmtiles[k][:, m * P:(m + 1) * P],
                                rhs=ytiles[k][:, n * CHUNK:(n + 1) * CHUNK],
                                start=(k == 0),
                                stop=(k == KT - 1),
                            )
                        nc.vector.tensor_reduce(
                            mzx[:, n:n + 1], P2,
                            axis=mybir.AxisListType.X, op=mybir.AluOpType.max)
                        # copy chunk out of PSUM with the bias folded in
                        nc.scalar.activation(
                            out=Z[:, n * CHUNK:(n + 1) * CHUNK], in_=P2,
                            func=mybir.ActivationFunctionType.Identity,
                            bias=b2sb[:, m:m + 1], scale=1.0)
                    m2 = st.tile([P, 1], F32)
                    nc.vector.tensor_reduce(
                        m2, mzx, axis=mybir.AxisListType.X, op=mybir.AluOpType.max)
                    nc.vector.tensor_tensor(
                        m2, m2, b2sb[:, m:m + 1], op=mybir.AluOpType.add)
                    mask2 = st.tile([P, 1], F32)
                    nc.vector.tensor_scalar(
                        out=mask2, in0=m2, scalar1=PW_THRESH, scalar2=None,
                        op0=mybir.AluOpType.is_ge)
                    # z = max(z*mask, 0): alternate DVE / ACT to balance engines
                    if m % 2 == 0:
                        nc.vector.tensor_scalar(
                            out=Z, in0=Z, scalar1=mask2, scalar2=0.0,
                            op0=mybir.AluOpType.mult, op1=mybir.AluOpType.max)
                    else:
                        nc.scalar.activation(
                            out=Z, in_=Z,
                            func=mybir.ActivationFunctionType.Relu,
                            bias=0.0, scale=mask2)
                    nc.sync.dma_start(
                        out=zs_flat[b, m * P:(m + 1) * P, :], in_=Z)
    nc.compile()
    return nc


def _prep_params(dw_w, dw_b, dw_gamma, dw_beta, dw_mean, dw_var,
                 pw_w, pw_b, pw_gamma, pw_beta, pw_mean, pw_var):
    dw_scale = dw_gamma / np.sqrt(dw_var + EPS)
    b1 = dw_b * dw_scale + dw_beta - dw_mean * dw_scale          # (256,)
    w1 = dw_w[:, 0] * dw_scale[:, None, None]                    # (256,3,3)

    dwdiag = np.zeros((P, KT, 9, P), np.float32)
    idx = np.arange(P)
    for k in range(KT):
        for t in range(9):
            dwdiag[idx, k, t, idx] = w1[k * P:(k + 1) * P, t // 3, t % 3]

    pw_scale = pw_gamma / np.sqrt(pw_var + EPS)
    b2 = pw_b * pw_scale + pw_beta - pw_mean * pw_scale          # (512,)
    w2 = pw_w * pw_scale[:, None]                                # (512,256)
    # w2t[ck, k, o] = w2[o, k*128+ck]
    w2t = np.ascontiguousarray(
        w2.T.reshape(KT, P, COUT).transpose(1, 0, 2)).astype(np.float32)
    b1t = np.ascontiguousarray(b1.reshape(KT, P).T).astype(np.float32)
    b2t = np.ascontiguousarray(b2.reshape(MT, P).T).astype(np.float32)
    return (np.ascontiguousarray(dwdiag).astype(ml_dtypes.bfloat16),
            w2t.astype(ml_dtypes.bfloat16), b1t, b2t)


def kernel(x, dw_w, dw_b, dw_gamma, dw_beta, dw_mean, dw_var,
           pw_w, pw_b, pw_gamma, pw_beta, pw_mean, pw_var):
    global _cached_nc
    x = np.ascontiguousarray(np.asarray(x, np.float32))
    args = [np.asarray(a, np.float32) for a in
            (dw_w, dw_b, dw_gamma, dw_beta, dw_mean, dw_var,
             pw_w, pw_b, pw_gamma, pw_beta, pw_mean, pw_var)]
    dwdiag, w2t, b1t, b2t = _prep_params(*args)
    x16 = x.astype(ml_dtypes.bfloat16)

    if _cached_nc is None:
        _cached_nc = _build_program()
    nc = _cached_nc

    in_maps = []
    for c in range(NCORES):
        in_maps.append({
            "xs": np.ascontiguousarray(x16[c * BPC:(c + 1) * BPC]),
            "dwdiag": dwdiag,
            "w2t": w2t,
            "b1t": b1t,
            "b2t": b2t,
        })
    res = run_bass_kernel_spmd(nc, in_maps, core_ids=list(range(NCORES)))
    out = np.concatenate([res.results[c]["zs"] for c in range(NCORES)], axis=0)
    return out


# revision 12
# speedup vs baseline: 1.0832x; 1.0831x over previous
"""Depthwise-separable conv2d block (dw3x3 + BN + ReLU + map-cut, pw1x1 + BN +
ReLU + map-cut) on 8 Trainium2 NeuronCores, data-parallel over the batch dim.

Fixed problem shapes: x (32,256,56,56) f32 -> out (32,512,54,54) f32.

Per-core device program (4 images each):
  - depthwise 3x3 VALID conv as 9 PSUM-accumulated diagonal matmuls per
    486-column chunk (fp32r, full PE rate at N>=256)
  - fused PSUM->SBUF copy + running per-map max via tensor_tensor_reduce
  - per-map cut mask + BN-folded bias + ReLU in one scalar-engine pass:
    y = relu(conv * mask + (bias * mask))
  - pointwise 1x1 conv as K=256 GEMM (fp32r), same fused max/copy, same
    fused mask+bias+ReLU pass, then DMA out.
BatchNorm (inference) is folded into the conv weights/biases on the host.
"""

import ml_dtypes
import numpy as np

import concourse.bacc as bacc
import concourse.bass as bass
import concourse.mybir as mybir
import concourse.tile as tile
from concourse.bass_utils import run_bass_kernel_spmd

EPS = 1e-5
DW_THRESH = 4.0
PW_THRESH = 0.001

B, CIN, COUT, H, W = 32, 256, 512, 56, 56
HO, WO = 54, 54
NPIX = HO * WO          # 2916
NCORES = 8
BPC = B // NCORES       # 4 images per core
P = 128                 # partitions
KT = CIN // P           # 2 cin tiles
MT = COUT // P          # 4 cout tiles
NCH = 6                 # output chunks per map: 6 x (9 rows x 54 cols)
CHROWS = HO // NCH      # 9
CHUNK = CHROWS * WO     # 486 columns per chunk (one PSUM bank)
NEG = -3.0e38

F32 = mybir.dt.float32
F32R = mybir.dt.float32r
BF16 = mybir.dt.bfloat16

_cached_nc = None


def _build_program():
    nc = bacc.Bacc("TRN2", target_bir_lowering=False, debug=False)

    xs = nc.dram_tensor("xs", [BPC, CIN, H, W], BF16, kind="ExternalInput").ap()
    dwdiag = nc.dram_tensor("dwdiag", [P, KT, 9, P], BF16, kind="ExternalInput").ap()
    w2t = nc.dram_tensor("w2t", [P, KT, COUT], BF16, kind="ExternalInput").ap()
    b1t = nc.dram_tensor("b1t", [P, KT], F32, kind="ExternalInput").ap()
    b2t = nc.dram_tensor("b2t", [P, MT], F32, kind="ExternalInput").ap()
    zs = nc.dram_tensor("zs", [BPC, COUT, HO, WO], F32, kind="ExternalOutput").ap()
    zs_flat = zs.rearrange("b c h w -> b c (h w)")

    HCH = NCH // 2          # 3 chunks per PSUM half-group
    BANK = 512              # fp32 elems per PSUM bank

    with tile.TileContext(nc) as tc:
        with (
            tc.tile_pool(name="consts", bufs=1) as consts,
            tc.tile_pool(name="xp", bufs=8) as xp,
            tc.tile_pool(name="yp", bufs=4) as yp,
            tc.tile_pool(name="zp", bufs=4) as zp,
            tc.tile_pool(name="wm", bufs=4) as wmp,
            tc.tile_pool(name="st", bufs=24) as st,
            tc.tile_pool(name="psdw", bufs=2, space="PSUM") as psdw,
            tc.tile_pool(name="pspw", bufs=2, space="PSUM") as pspw,
        ):
            # first image's inputs and the dw weights gate the first matmul —
            # issue those DMAs first, the rest of the prefetch afterwards
            xtiles = {}
            for b, k in [(0, 0), (0, 1)]:
                X = xp.tile([P, H, W], BF16)
                nc.sync.dma_start(out=X, in_=xs[b, k * P:(k + 1) * P, :, :])
                xtiles[b, k] = X
            dwsb = consts.tile([P, KT, 9, P], BF16)
            nc.sync.dma_start(out=dwsb, in_=dwdiag)
            w2sb = consts.tile([P, KT, COUT], BF16)
            nc.sync.dma_start(out=w2sb, in_=w2t)
            b1sb = consts.tile([P, KT], F32)
            nc.sync.dma_start(out=b1sb, in_=b1t)
            b2sb = consts.tile([P, MT], F32)
            nc.sync.dma_start(out=b2sb, in_=b2t)
            for b in range(BPC):
                for k in range(KT):
                    if (b, k) in xtiles:
                        continue
                    X = xp.tile([P, H, W], BF16)
                    nc.sync.dma_start(out=X, in_=xs[b, k * P:(k + 1) * P, :, :])
                    xtiles[b, k] = X

            for b in range(BPC):
                ytiles = []
                wmtiles = []
                for k in range(KT):
                    X = xtiles[b, k]

                    Y = yp.tile([P, NPIX], BF16)
                    Y3 = Y.rearrange("p (c x) -> p c x", x=CHUNK)
                    for h in range(2):
                        # 3 PSUM banks hold 3 conv chunks (27 rows of output)
                        P1 = psdw.tile([P, HCH, BANK], F32)
                        for j in range(HCH):
                            n = h * HCH + j
                            for t in range(9):
                                di, dj = t // 3, t % 3
                                rhs = X[:,
                                        CHROWS * n + di: CHROWS * n + di + CHROWS,
                                        dj: dj + WO]
                                nc.tensor.matmul(
                                    P1[:, j, 0:CHUNK],
                                    lhsT=dwsb[:, k, t, :],
                                    rhs=rhs,
                                    start=(t == 0),
                                    stop=(t == 8),
                                )
                        # mask-free relu+bias straight away: PSUM frees early.
                        # the dw cut is applied later via masked pw weights.
                        nc.scalar.activation(
                            out=Y3[:, h * HCH:(h + 1) * HCH, :],
                            in_=P1[:, :, 0:CHUNK],
                            func=mybir.ActivationFunctionType.Relu,
                            bias=b1sb[:, k:k + 1], scale=1.0)
                    # mask off the PSUM-release path: max(relu(conv+b1)) >= 4
                    # is equivalent to max(conv)+b1 >= 4 since the cut is > 0
                    m1 = st.tile([P, 1], F32)
                    nc.vector.tensor_reduce(
                        m1, Y, axis=mybir.AxisListType.X, op=mybir.AluOpType.max)
                    mask1 = st.tile([P, 1], F32)
                    nc.vector.tensor_scalar(
                        out=mask1, in0=m1, scalar1=DW_THRESH, scalar2=None,
                        op0=mybir.AluOpType.is_ge)
                    # fold the per-(image,channel) dw cut into the pw weights
                    w2m = wmp.tile([P, COUT], BF16)
                    nc.vector.tensor_scalar(
                        out=w2m, in0=w2sb[:, k, :], scalar1=mask1, scalar2=None,
                        op0=mybir.AluOpType.mult)
                    ytiles.append(Y)
                    wmtiles.append(w2m)

                for m in range(MT):
                    Z = zp.tile([P, NPIX], F32)
                    for n in range(NCH):
                        P2 = pspw.tile([P, CHUNK], F32)
                        for k in range(KT):
                            nc.tensor.matmul(
                                P2,
                                lhsT=wmtiles[k][:, m * P:(m + 1) * P],
                                rhs=ytiles[k][:, n * CHUNK:(n + 1) * CHUNK],
                                start=(k == 0),
                                stop=(k == KT - 1),
                            )
                        # copy chunk out of PSUM with the bias folded in
                        nc.scalar.activation(
                            out=Z[:, n * CHUNK:(n + 1) * CHUNK], in_=P2,
                            func=mybir.ActivationFunctionType.Identity,
                            bias=b2sb[:, m:m + 1], scale=1.0)
                    m2 = st.tile([P, 1], F32)
                    nc.vector.tensor_reduce(
                        m2, Z, axis=mybir.AxisListType.X, op=mybir.AluOpType.max)
                    mask2 = st.tile([P, 1], F32)
                    nc.vector.tensor_scalar(
                        out=mask2, in0=m2, scalar1=PW_THRESH, scalar2=None,
                        op0=mybir.AluOpType.is_ge)
                    # z = max(z*mask, 0): alternate DVE / ACT to balance engines
                    if m % 2 == 0:
                        nc.vector.tensor_scalar(
                            out=Z, in0=Z, scalar1=mask2, scalar2=0.0,
                            op0=mybir.AluOpType.mult, op1=mybir.AluOpType.max)
                    else:
                        nc.scalar.activation(
                            out=Z, in_=Z,
                            func=mybir.ActivationFunctionType.Relu,
                            bias=0.0, scale=mask2)
                    nc.sync.dma_start(
                        out=zs_flat[b, m * P:(m + 1) * P, :], in_=Z)
    nc.compile()
    return nc


def _prep_params(dw_w, dw_b, dw_gamma, dw_beta, dw_mean, dw_var,
                 pw_w, pw_b, pw_gamma, pw_beta, pw_mean, pw_var):
    dw_scale = dw_gamma / np.sqrt(dw_var + EPS)
    b1 = dw_b * dw_scale + dw_beta - dw_mean * dw_scale          # (256,)
    w1 = dw_w[:, 0] * dw_scale[:, None, None]                    # (256,3,3)

    dwdiag = np.zeros((P, KT, 9, P), np.float32)
    idx = np.arange(P)
    for k in range(KT):
        for t in range(9):
            dwdiag[idx, k, t, idx] = w1[k * P:(k + 1) * P, t // 3, t % 3]

    pw_scale = pw_gamma / np.sqrt(pw_var + EPS)
    b2 = pw_b * pw_scale + pw_beta - pw_mean * pw_scale          # (512,)
    w2 = pw_w * pw_scale[:, None]                                # (512,256)
    # w2t[ck, k, o] = w2[o, k*128+ck]
    w2t = np.ascontiguousarray(
        w2.T.reshape(KT, P, COUT).transpose(1, 0, 2)).astype(np.float32)
    b1t = np.ascontiguousarray(b1.reshape(KT, P).T).astype(np.float32)
    b2t = np.ascontiguousarray(b2.reshape(MT, P).T).astype(np.float32)
    return (np.ascontiguousarray(dwdiag).astype(ml_dtypes.bfloat16),
            w2t.astype(ml_dtypes.bfloat16), b1t, b2t)


def kernel(x, dw_w, dw_b, dw_gamma, dw_beta, dw_mean, dw_var,
           pw_w, pw_b, pw_gamma, pw_beta, pw_mean, pw_var):
    global _cached_nc
    x = np.ascontiguousarray(np.asarray(x, np.float32))
    args = [np.asarray(a, np.float32) for a in
            (dw_w, dw_b, dw_gamma, dw_beta, dw_mean, dw_var,
             pw_w, pw_b, pw_gamma, pw_beta, pw_mean, pw_var)]
    dwdiag, w2t, b1t, b2t = _prep_params(*args)
    x16 = x.astype(ml_dtypes.bfloat16)

    if _cached_nc is None:
        _cached_nc = _build_program()
    nc = _cached_nc

    in_maps = []
    for c in range(NCORES):
        in_maps.append({
            "xs": np.ascontiguousarray(x16[c * BPC:(c + 1) * BPC]),
            "dwdiag": dwdiag,
            "w2t": w2t,
            "b1t": b1t,
            "b2t": b2t,
        })
    res = run_bass_kernel_spmd(nc, in_maps, core_ids=list(range(NCORES)))
    out = np.concatenate([res.results[c]["zs"] for c in range(NCORES)], axis=0)
    return out


# revision 15
# speedup vs baseline: 1.0977x; 1.0134x over previous
"""Depthwise-separable conv2d block (dw3x3 + BN + ReLU + map-cut, pw1x1 + BN +
ReLU + map-cut) on 8 Trainium2 NeuronCores, data-parallel over the batch dim.

Fixed problem shapes: x (32,256,56,56) f32 -> out (32,512,54,54) f32.

Per-core device program (4 images each):
  - depthwise 3x3 VALID conv as 9 PSUM-accumulated diagonal matmuls per
    486-column chunk (fp32r, full PE rate at N>=256)
  - fused PSUM->SBUF copy + running per-map max via tensor_tensor_reduce
  - per-map cut mask + BN-folded bias + ReLU in one scalar-engine pass:
    y = relu(conv * mask + (bias * mask))
  - pointwise 1x1 conv as K=256 GEMM (fp32r), same fused max/copy, same
    fused mask+bias+ReLU pass, then DMA out.
BatchNorm (inference) is folded into the conv weights/biases on the host.
"""

import ml_dtypes
import numpy as np

import concourse.bacc as bacc
import concourse.bass as bass
import concourse.mybir as mybir
import concourse.tile as tile
from concourse.bass_utils import run_bass_kernel_spmd

EPS = 1e-5
DW_THRESH = 4.0
PW_THRESH = 0.001

B, CIN, COUT, H, W = 32, 256, 512, 56, 56
HO, WO = 54, 54
NPIX = HO * WO          # 2916
NCORES = 8
BPC = B // NCORES       # 4 images per core
P = 128                 # partitions
KT = CIN // P           # 2 cin tiles
MT = COUT // P          # 4 cout tiles
NCH = 6                 # output chunks per map: 6 x (9 rows x 54 cols)
CHROWS = HO // NCH      # 9
CHUNK = CHROWS * WO     # 486 columns per chunk (one PSUM bank)
NEG = -3.0e38

F32 = mybir.dt.float32
F32R = mybir.dt.float32r
BF16 = mybir.dt.bfloat16

_cached_nc = None


def _build_program():
    nc = bacc.Bacc("TRN2", target_bir_lowering=False, debug=False)

    xs = nc.dram_tensor("xs", [BPC, CIN, H, W], BF16, kind="ExternalInput").ap()
    dwdiag = nc.dram_tensor("dwdiag", [P, KT, 9, P], BF16, kind="ExternalInput").ap()
    w2t = nc.dram_tensor("w2t", [P, KT, COUT], BF16, kind="ExternalInput").ap()
    b1t = nc.dram_tensor("b1t", [P, KT], F32, kind="ExternalInput").ap()
    b2t = nc.dram_tensor("b2t", [P, MT], F32, kind="ExternalInput").ap()
    zs = nc.dram_tensor("zs", [BPC, COUT, HO, WO], F32, kind="ExternalOutput").ap()
    zs_flat = zs.rearrange("b c h w -> b c (h w)")

    HCH = NCH // 2          # 3 chunks per PSUM half-group
    BANK = 512              # fp32 elems per PSUM bank

    with tile.TileContext(nc) as tc:
        with (
            tc.tile_pool(name="consts", bufs=1) as consts,
            tc.tile_pool(name="xp", bufs=8) as xp,
            tc.tile_pool(name="yp", bufs=4) as yp,
            tc.tile_pool(name="zp", bufs=4) as zp,
            tc.tile_pool(name="wm", bufs=4) as wmp,
            tc.tile_pool(name="st", bufs=24) as st,
            tc.tile_pool(name="psdw", bufs=2, space="PSUM") as psdw,
            tc.tile_pool(name="pspw", bufs=2, space="PSUM") as pspw,
        ):
            # first image's inputs and the dw weights gate the first matmul —
            # issue those DMAs first, the rest of the prefetch afterwards
            xtiles = {}
            X = xp.tile([P, H, W], BF16)
            nc.sync.dma_start(out=X, in_=xs[0, 0:P, :, :])
            xtiles[0, 0] = X
            dwsb = consts.tile([P, KT, 9, P], BF16)
            nc.sync.dma_start(out=dwsb[:, 0], in_=dwdiag[:, 0])
            b1sb = consts.tile([P, KT], F32)
            nc.sync.dma_start(out=b1sb, in_=b1t)
            X = xp.tile([P, H, W], BF16)
            nc.sync.dma_start(out=X, in_=xs[0, P:2 * P, :, :])
            xtiles[0, 1] = X
            nc.sync.dma_start(out=dwsb[:, 1], in_=dwdiag[:, 1])
            w2sb = consts.tile([P, KT, COUT], BF16)
            nc.sync.dma_start(out=w2sb, in_=w2t)
            b2sb = consts.tile([P, MT], F32)
            nc.sync.dma_start(out=b2sb, in_=b2t)
            for b in range(BPC):
                for k in range(KT):
                    if (b, k) in xtiles:
                        continue
                    X = xp.tile([P, H, W], BF16)
                    nc.sync.dma_start(out=X, in_=xs[b, k * P:(k + 1) * P, :, :])
                    xtiles[b, k] = X

            for b in range(BPC):
                ytiles = []
                wmtiles = []
                for k in range(KT):
                    X = xtiles[b, k]

                    Y = yp.tile([P, NPIX], BF16)
                    Y3 = Y.rearrange("p (c x) -> p c x", x=CHUNK)
                    for h in range(2):
                        # 3 PSUM banks hold 3 conv chunks (27 rows of output)
                        P1 = psdw.tile([P, HCH, BANK], F32)
                        for j in range(HCH):
                            n = h * HCH + j
                            for t in range(9):
                                di, dj = t // 3, t % 3
                                rhs = X[:,
                                        CHROWS * n + di: CHROWS * n + di + CHROWS,
                                        dj: dj + WO]
                                nc.tensor.matmul(
                                    P1[:, j, 0:CHUNK],
                                    lhsT=dwsb[:, k, t, :],
                                    rhs=rhs,
                                    start=(t == 0),
                                    stop=(t == 8),
                                )
                        # mask-free relu+bias straight away: PSUM frees early.
                        # the dw cut is applied later via masked pw weights.
                        nc.scalar.activation(
                            out=Y3[:, h * HCH:(h + 1) * HCH, :],
                            in_=P1[:, :, 0:CHUNK],
                            func=mybir.ActivationFunctionType.Relu,
                            bias=b1sb[:, k:k + 1], scale=1.0)
                    # mask off the PSUM-release path: max(relu(conv+b1)) >= 4
                    # is equivalent to max(conv)+b1 >= 4 since the cut is > 0
                    m1 = st.tile([P, 1], F32)
                    nc.vector.tensor_reduce(
                        m1, Y, axis=mybir.AxisListType.X, op=mybir.AluOpType.max)
                    mask1 = st.tile([P, 1], F32)
                    nc.vector.tensor_scalar(
                        out=mask1, in0=m1, scalar1=DW_THRESH, scalar2=None,
                        op0=mybir.AluOpType.is_ge)
                    # fold the per-(image,channel) dw cut into the pw weights
                    w2m = wmp.tile([P, COUT], BF16)
                    nc.vector.tensor_scalar(
                        out=w2m, in0=w2sb[:, k, :], scalar1=mask1, scalar2=None,
                        op0=mybir.AluOpType.mult)
                    ytiles.append(Y)
                    wmtiles.append(w2m)

                last = (b == BPC - 1)
                for m in range(MT):
                    Z = zp.tile([P, NPIX], F32)
                    mzx = st.tile([P, NCH], F32, name="mzx", tag="mzx") if last else None
                    for n in range(NCH):
                        P2 = pspw.tile([P, CHUNK], F32)
                        for k in range(KT):
                            nc.tensor.matmul(
                                P2,
                                lhsT=wmtiles[k][:, m * P:(m + 1) * P],
                                rhs=ytiles[k][:, n * CHUNK:(n + 1) * CHUNK],
                                start=(k == 0),
                                stop=(k == KT - 1),
                            )
                        if last:
                            # per-chunk maxes so mask2 is ready right after
                            # the last matmul (shortens the kernel tail)
                            nc.vector.tensor_reduce(
                                mzx[:, n:n + 1], P2,
                                axis=mybir.AxisListType.X,
                                op=mybir.AluOpType.max)
                        # copy chunk out of PSUM with the bias folded in
                        nc.scalar.activation(
                            out=Z[:, n * CHUNK:(n + 1) * CHUNK], in_=P2,
                            func=mybir.ActivationFunctionType.Identity,
                            bias=b2sb[:, m:m + 1], scale=1.0)
                    m2 = st.tile([P, 1], F32)
                    if last:
                        nc.vector.tensor_reduce(
                            m2, mzx, axis=mybir.AxisListType.X,
                            op=mybir.AluOpType.max)
                        nc.vector.tensor_tensor(
                            m2, m2, b2sb[:, m:m + 1], op=mybir.AluOpType.add)
                    else:
                        nc.vector.tensor_reduce(
                            m2, Z, axis=mybir.AxisListType.X,
                            op=mybir.AluOpType.max)
                    mask2 = st.tile([P, 1], F32)
                    nc.vector.tensor_scalar(
                        out=mask2, in0=m2, scalar1=PW_THRESH, scalar2=None,
                        op0=mybir.AluOpType.is_ge)
                    # z = max(z*mask, 0): split halves so the store overlaps,
                    # alternating DVE / ACT to balance engines
                    HP = NPIX // 2
                    for h0 in range(2):
                        zslice = Z[:, h0 * HP:(h0 + 1) * HP]
                        if (m + h0) % 2 == 0:
                            nc.vector.tensor_scalar(
                                out=zslice, in0=zslice, scalar1=mask2,
                                scalar2=0.0, op0=mybir.AluOpType.mult,
                                op1=mybir.AluOpType.max)
                        else:
                            nc.scalar.activation(
                                out=zslice, in_=zslice,
                                func=mybir.ActivationFunctionType.Relu,
                                bias=0.0, scale=mask2)
                        nc.sync.dma_start(
                            out=zs_flat[b, m * P:(m + 1) * P,
                                        h0 * HP:(h0 + 1) * HP],
                            in_=zslice)
    nc.compile()
    return nc


def _prep_params(dw_w, dw_b, dw_gamma, dw_beta, dw_mean, dw_var,
                 pw_w, pw_b, pw_gamma, pw_beta, pw_mean, pw_var):
    dw_scale = dw_gamma / np.sqrt(dw_var + EPS)
    b1 = dw_b * dw_scale + dw_beta - dw_mean * dw_scale          # (256,)
    w1 = dw_w[:, 0] * dw_scale[:, None, None]                    # (256,3,3)

    dwdiag = np.zeros((P, KT, 9, P), np.float32)
    idx = np.arange(P)
    for k in range(KT):
        for t in range(9):
            dwdiag[idx, k, t, idx] = w1[k * P:(k + 1) * P, t // 3, t % 3]

    pw_scale = pw_gamma / np.sqrt(pw_var + EPS)
    b2 = pw_b * pw_scale + pw_beta - pw_mean * pw_scale          # (512,)
    w2 = pw_w * pw_scale[:, None]                                # (512,256)
    # w2t[ck, k, o] = w2[o, k*128+ck]
    w2t = np.ascontiguousarray(
        w2.T.reshape(KT, P, COUT).transpose(1, 0, 2)).astype(np.float32)
    b1t = np.ascontiguousarray(b1.reshape(KT, P).T).astype(np.float32)
    b2t = np.ascontiguousarray(b2.reshape(MT, P).T).astype(np.float32)
    return (np.ascontiguousarray(dwdiag).astype(ml_dtypes.bfloat16),
            w2t.astype(ml_dtypes.bfloat16), b1t, b2t)


def kernel(x, dw_w, dw_b, dw_gamma, dw_beta, dw_mean, dw_var,
           pw_w, pw_b, pw_gamma, pw_beta, pw_mean, pw_var):
    global _cached_nc
    x = np.ascontiguousarray(np.asarray(x, np.float32))
    args = [np.asarray(a, np.float32) for a in
            (dw_w, dw_b, dw_gamma, dw_beta, dw_mean, dw_var,
             pw_w, pw_b, pw_gamma, pw_beta, pw_mean, pw_var)]
    dwdiag, w2t, b1t, b2t = _prep_params(*args)
    x16 = x.astype(ml_dtypes.bfloat16)

    if _cached_nc is None:
        _cached_nc = _build_program()
    nc = _cached_nc

    in_maps = []
    for c in range(NCORES):
        in_maps.append({
            "xs": np.ascontiguousarray(x16[c * BPC:(c + 1) * BPC]),
            "dwdiag": dwdiag,
            "w2t": w2t,
            "b1t": b1t,
            "b2t": b2t,
        })
    res = run_bass_kernel_spmd(nc, in_maps, core_ids=list(range(NCORES)))
    out = np.concatenate([res.results[c]["zs"] for c in range(NCORES)], axis=0)
    return out
